# revision 41
# baseline (speedup 1.0000x reference)
"""DirectionalAttention Trainium2 kernel.

Math (per batch b, query node i, key node j):
    Q/K/V = features @ W{q,k,v} + b{q,k,v}                     [B,N,H]
    dir_enc[b,i,j] = (loc_j - loc_i) @ Wd[0:2] + wind_bi @ Wd[2:4] + bd
    pre[b,i,j,:]  = (Q_bi * K_bj) @ W1a + dir_enc @ W1b + b1   [MLP]
    scores[b,i,j] = relu(pre) @ W2 (+ b2, softmax-invariant -> dropped)
    out[b,i]      = softmax_j(scores) @ V_b

Key restructuring: dir_enc is rank-2 in j (loc has 2 coords), so with
    WdW1   = Wd[0:2] @ W1b   [2,MLP]
    cw1_bi = (-loc_i @ Wd[0:2] + wind_bi @ Wd[2:4] + bd) @ W1b + b1
the whole pre-activation for one i collapses to ONE 66-row matmul:
    pre[m, j] = concat(Q_bi*W1a, WdW1).T @ concat(K_b.T, loc.T)  + cw1_bi[m]
with the per-i bias applied inside the relu (per-partition bias).

Sharding: query axis i split across 8 cores (64 i's per batch per core);
K/V/params replicated; softmax over j stays local. No collectives.
"""

import os
from contextlib import ExitStack

import numpy as np

B, N, H, MLP = 4, 512, 64, 128
NCORES = 8
ISH = N // NCORES            # i's per batch per core = 64
SLOTS = B * ISH              # (b, i) pairs per core = 256
BLK = 128                    # slots per softmax block
NBLK = SLOTS // BLK          # 2
CP = 128                     # padded contraction dim: all matmuls share C=128
DELAY = 8                    # slots the scores matmul trails the main matmul

SD_NAME = os.environ.get("DIRATTN_SD", "bf16")        # hmid/scores-mm dtype
ACTP_N = int(os.environ.get("DIRATTN_ACTP", "5"))    # of 8 relu-pairs on ScalarE
PD_NAME = os.environ.get("DIRATTN_PD", "bf16")       # softmax-P / attn@V dtype
EPI_DELAY = 10               # mains of next block issued before epilogue

_prog_cache = {}


def _build_program():
    import concourse.bass as bass
    import concourse.tile as tile
    from concourse import bacc, mybir
    from concourse.masks import make_identity
    from concourse.tile import add_dep_helper

    f32 = mybir.dt.float32
    SD = mybir.dt.bfloat16 if SD_NAME == "bf16" else f32
    PD = mybir.dt.bfloat16 if PD_NAME == "bf16" else f32
    AF = mybir.ActivationFunctionType
    AL = mybir.AluOpType

    nc = bacc.Bacc("TRN2", target_bir_lowering=False, debug=False,
                   enable_asserts=False)

    d_ft1 = nc.dram_tensor("ft1", (H + 1, B * N), SD, kind="ExternalInput").ap()
    d_fq1 = nc.dram_tensor("fq1", (H + 1, SLOTS), SD, kind="ExternalInput").ap()
    d_locT = nc.dram_tensor("locT", (2, N), f32, kind="ExternalInput").ap()
    d_rhs5 = nc.dram_tensor("rhs5", (5, SLOTS), f32, kind="ExternalInput").ap()
    d_wq1 = nc.dram_tensor("wq1", (H + 1, H), SD, kind="ExternalInput").ap()
    d_wk1 = nc.dram_tensor("wk1", (H + 1, H), SD, kind="ExternalInput").ap()
    d_wv1 = nc.dram_tensor("wv1", (H + 1, H), SD, kind="ExternalInput").ap()
    d_l5 = nc.dram_tensor("lhsT5", (5, MLP), f32, kind="ExternalInput").ap()
    d_w1ax = nc.dram_tensor("w1ax", (CP, MLP), SD, kind="ExternalInput").ap()
    d_w2oh = nc.dram_tensor("w2oh", (MLP, 2 * BLK - 1), SD, kind="ExternalInput").ap()
    d_out = nc.dram_tensor("out", (B, ISH, H), f32, kind="ExternalOutput").ap()

    with tile.TileContext(nc) as tc, ExitStack() as ctx:
        singles = ctx.enter_context(tc.tile_pool(name="singles", bufs=1))
        mm_pool = ctx.enter_context(tc.tile_pool(name="mm", bufs=4, space="PSUM"))
        sc_pool = ctx.enter_context(tc.tile_pool(name="scps", bufs=2, space="PSUM"))
        tr_pool = ctx.enter_context(tc.tile_pool(name="trps", bufs=1, space="PSUM"))
        ov_pool = ctx.enter_context(tc.tile_pool(name="ovps", bufs=1, space="PSUM"))
        lt_pool = ctx.enter_context(tc.tile_pool(name="lt", bufs=7))
        hm_pool = ctx.enter_context(tc.tile_pool(name="hm", bufs=14))
        soft = ctx.enter_context(tc.tile_pool(name="soft", bufs=2))

        def load(name, dram, shape, dt=f32, eng=None):
            t = singles.tile(list(shape), dt, tag=name)
            (eng or nc.sync).dma_start(out=t[:], in_=dram)
            return t

        # split loads across the two hwdge queues; first-needed first
        wk1 = load("wk1", d_wk1, (H + 1, H), SD)
        wq1 = load("wq1", d_wq1, (H + 1, H), SD, eng=nc.scalar)
        ft1 = singles.tile([H + 1, B * N], SD, tag="ft1")
        for fb in range(B):
            nc.sync.dma_start(out=ft1[:, fb * N:(fb + 1) * N],
                              in_=d_ft1[:, fb * N:(fb + 1) * N])
        fq1 = load("fq1", d_fq1, (H + 1, SLOTS), SD, eng=nc.scalar)
        wv1 = load("wv1", d_wv1, (H + 1, H), SD)
        locT = load("locT", d_locT, (2, N), eng=nc.scalar)
        rhs5 = load("rhs5", d_rhs5, (5, SLOTS), eng=nc.scalar)
        l5 = load("l5", d_l5, (5, MLP), eng=nc.scalar)
        w1ax = load("w1ax", d_w1ax, (CP, MLP), SD, eng=nc.scalar)
        w2oh = load("w2oh", d_w2oh, (MLP, 2 * BLK - 1), SD)

        ident = singles.tile([128, 128], PD, tag="ident")
        make_identity(nc, ident[:])

        # --- prep: K^T rows + [1,1,locx,locy] rows per batch -> rhs_b
        # [128, 512].  Row 64 is all-ones and pairs with the per-slot cw1
        # bias row of lt, so the relu needs no bias operand. ---
        rhs_b = []
        for b in range(B):
            rb = singles.tile([CP, N], SD, tag=f"rhsb{b}")
            ps = mm_pool.tile([MLP, N], f32, name="ps", tag="mm")
            nc.tensor.matmul(ps[0:H, :], lhsT=wk1[:],
                             rhs=ft1[:, b * N:(b + 1) * N],
                             start=True, stop=True)
            nc.vector.memset(rb[H:CP, :], 0.0)
            nc.vector.memset(rb[H:H + 1, :], 1.0)
            nc.vector.tensor_copy(rb[0:H, :], ps[0:H, :])
            nc.scalar.copy(rb[96:98, :], locT[:])
            rhs_b.append(rb)

        # --- prep: Q^T for own shard (+ones rows 64-127) -> qt1 [128, 256] ---
        qt1 = singles.tile([CP, SLOTS], f32, tag="qt1")
        nc.vector.memset(qt1[H:CP, :], 1.0)
        psq = mm_pool.tile([MLP, N], f32, name="psq", tag="mm")
        nc.tensor.matmul(psq[0:H, 0:SLOTS], lhsT=wq1[:], rhs=fq1[:],
                         start=True, stop=True)
        nc.vector.tensor_copy(qt1[0:H, :], psq[0:H, 0:SLOTS])

        # --- prep: V_b as 4x [128 j, 64 h] tiles per batch ---
        vsb = {}
        for b in range(B):
            for jc in range(4):
                vt = singles.tile([128, H], PD, tag=f"v{b}_{jc}")
                ps = mm_pool.tile([MLP, N], f32, name="ps", tag="mm")
                nc.tensor.matmul(
                    ps[:, 0:H],
                    lhsT=ft1[:, b * N + jc * 128: b * N + (jc + 1) * 128],
                    rhs=wv1[:], start=True, stop=True)
                nc.vector.tensor_copy(vt[:], ps[:, 0:H])
                vsb[(b, jc)] = vt

        # --- prep: per-(b,i) relu bias cw1 [128 m, 256 slots] ---
        cw1 = singles.tile([MLP, SLOTS], f32, tag="cw1")
        psc = mm_pool.tile([MLP, N], f32, name="psc", tag="mm")
        nc.tensor.matmul(psc[:, 0:SLOTS], lhsT=l5[:], rhs=rhs5[:],
                         start=True, stop=True)
        nc.vector.tensor_copy(cw1[:], psc[:, 0:SLOTS])

        # --- main loop (flat over all 256 slots, software-pipelined) ---
        # The scores matmul for slot p issues after the main matmul for
        # slot p+DELAY, so PE never stalls on the relu.  Each block's
        # softmax+attn@V epilogue is issued EPI_DELAY mains into the next
        # block so the PE keeps streaming mains while DVE does softmax.
        hms = {}
        mains = {}
        scoreses = {}
        scts = {}
        Ss = {}
        lts = {}

        def issue_scores(g):
            blk, p = g // BLK, g % BLK
            si = nc.tensor.matmul(scts[blk][:],
                                  lhsT=w2oh[:, BLK - 1 - p: 2 * BLK - 1 - p],
                                  rhs=hms.pop(g)[:],
                                  start=(p == 0), stop=(p == BLK - 1))
            scoreses[g] = si
            if g + DELAY in mains:
                add_dep_helper(si.ins, mains[g + DELAY].ins, sync=False,
                               reason="pipeline: scores_p after main_p+D")
            if p == BLK - 1:
                nc.vector.tensor_copy(Ss[blk][:], scts[blk][:])

        def build_lt(g):
            lt = lt_pool.tile([CP, MLP], SD, tag="lt")
            nc.vector.tensor_scalar_mul(lt[:], w1ax[:], qt1[:, g:g + 1])
            lts[g] = lt

        def epilogue(blk):
            # softmax over j for 128 (b,i) rows; P left unnormalized and
            # the 1/sum folded into the output copy (per-i scale).
            S = Ss.pop(blk)
            nmx = soft.tile([BLK, 1], f32, tag="nmx")
            nc.vector.tensor_reduce(nmx[:], S[:], axis=mybir.AxisListType.X,
                                    op=AL.max, negate=True)
            P = soft.tile([BLK, N], PD, tag="P")
            sm = soft.tile([BLK, 1], f32, tag="sm")
            nc.scalar.activation(P[:], S[:], AF.Exp, bias=nmx[:, 0:1],
                                 scale=1.0, accum_out=sm[:])
            rs = soft.tile([BLK, 1], f32, tag="rs")
            nc.vector.reciprocal(rs[:], sm[:])

            # attn @ V: transpose P, contract over j, scale by 1/sum
            atT = []
            for jc in range(4):
                tp = tr_pool.tile([128, 128], PD, tag="tr")
                nc.tensor.transpose(tp[:], P[:, jc * 128:(jc + 1) * 128],
                                    ident[:])
                ts = soft.tile([128, 128], PD, tag=f"atT{jc}")
                nc.vector.tensor_copy(ts[:], tp[:])
                atT.append(ts)
            for h2 in range(2):
                b = blk * 2 + h2
                ov = ov_pool.tile([ISH, H], f32, tag="ov")
                for jc in range(4):
                    nc.tensor.matmul(ov[:],
                                     lhsT=atT[jc][:, h2 * ISH:(h2 + 1) * ISH],
                                     rhs=vsb[(b, jc)][:],
                                     start=(jc == 0), stop=(jc == 3))
                ob = soft.tile([ISH, H], f32, tag="ob")
                nc.vector.tensor_scalar_mul(ob[:], ov[:],
                                            rs[h2 * ISH:(h2 + 1) * ISH, 0:1])
                nc.sync.dma_start(out=d_out[b, :, :], in_=ob[:])

        LOOKA = 4
        for g in range(LOOKA):
            build_lt(g)
        for g in range(SLOTS):
            blk, p = g // BLK, g % BLK
            b = g // ISH
            if p == 0:
                Ss[blk] = soft.tile([BLK, N], f32, name="S", tag="S")
                scts[blk] = sc_pool.tile([BLK, N], f32, name="sc", tag="sc")
            if g + LOOKA < SLOTS:
                build_lt(g + LOOKA)
            lt = lts.pop(g)

            ps = mm_pool.tile([MLP, N], f32, name="ps", tag="mm")
            mi = nc.tensor.matmul(ps[:], lhsT=lt[:], rhs=rhs_b[b][:],
                                  start=True, stop=True)
            mains[g] = mi
            if g - DELAY - 1 in scoreses:
                add_dep_helper(mi.ins, scoreses[g - DELAY - 1].ins,
                               sync=False,
                               reason="pipeline: main_p after scores")

            hm = hm_pool.tile([MLP, N], SD, name="hm", tag="hm")
            if p % 8 < ACTP_N:
                nc.scalar.activation(hm[:], ps[:], AF.Relu,
                                     bias=cw1[:, g:g + 1], scale=1.0)
            else:
                nc.vector.tensor_scalar(hm[:], ps[:], cw1[:, g:g + 1], 0.0,
                                        AL.add, AL.max)
            hms[g] = hm
            if g >= DELAY:
                issue_scores(g - DELAY)
            if p == EPI_DELAY and blk > 0:
                epilogue(blk - 1)
        for q in range(SLOTS - DELAY, SLOTS):
            issue_scores(q)
        epilogue(NBLK - 1)

    nc.compile()
    return nc


def _host_prep(features, wind_data, loc_feature, Wq, bq, Wk, bk, Wv, bv,
               Wd, bd, W1, b1, W2, b2):
    f32 = np.float32
    f = np.ascontiguousarray(features, dtype=f32)
    W1a, W1b = W1[:H].astype(f32), W1[H:].astype(f32)
    WdW1 = (Wd[0:2].astype(f32) @ W1b)
    Wd24W1 = (Wd[2:4].astype(f32) @ W1b)
    g = bd.astype(f32) @ W1b + b1.astype(f32)

    fT = f.reshape(B * N, H).T
    ones = np.ones((1, B * N), f32)
    ft1 = np.ascontiguousarray(np.vstack([fT, ones]))
    locT = np.ascontiguousarray(loc_feature.T.astype(f32))
    wind = np.asarray(wind_data, dtype=f32)

    if SD_NAME == "bf16":
        import ml_dtypes
        sd = ml_dtypes.bfloat16
    else:
        sd = f32
    ft1_sd = ft1.astype(sd)
    w2oh = np.zeros((MLP, 2 * BLK - 1), f32)
    w2oh[:, BLK - 1] = W2[:, 0]
    w2oh = w2oh.astype(sd)

    shared = {
        "ft1": ft1_sd,
        "locT": locT,
        "wq1": np.ascontiguousarray(np.vstack([Wq, bq[None]]).astype(sd)),
        "wk1": np.ascontiguousarray(np.vstack([Wk, bk[None]]).astype(sd)),
        "wv1": np.ascontiguousarray(np.vstack([Wv, bv[None]]).astype(sd)),
        "lhsT5": np.ascontiguousarray(
            np.vstack([-WdW1, Wd24W1, g[None]]).astype(f32)),
        "w1ax": np.ascontiguousarray(np.vstack(
            [W1a, np.zeros((32, MLP), f32), WdW1,
             np.zeros((CP - 98, MLP), f32)]).astype(sd)),
        "w2oh": w2oh,
    }
    in_maps = []
    for c in range(NCORES):
        i0, i1 = c * ISH, (c + 1) * ISH
        fq1 = np.concatenate(
            [ft1[:, b * N + i0: b * N + i1] for b in range(B)], axis=1)
        rhs5 = np.vstack([
            np.concatenate([locT[:, i0:i1]] * B, axis=1),
            np.concatenate([wind[b, i0:i1, :].T for b in range(B)], axis=1),
            np.ones((1, SLOTS), f32),
        ])
        in_maps.append({**shared,
                        "fq1": np.ascontiguousarray(fq1.astype(sd)),
                        "rhs5": np.ascontiguousarray(rhs5.astype(f32))})
    return in_maps


last_results = None


def _install_ntff_hook():
    """Provide antenv.axon_hooks (absent in this image) so that
    run_bass_kernel_spmd(trace=True) can capture NTFF profiles via the
    injected libaxon_pjrt.so C ABI."""
    import sys
    if "antenv.axon_hooks" in sys.modules:
        return
    import contextlib
    import ctypes
    import types

    so_path = "/opt/axon/libaxon_pjrt.so"
    try:
        lib = ctypes.CDLL(so_path)
        lib.axon_start_nrt_profile
    except (OSError, AttributeError):
        return
    lib.axon_start_nrt_profile.argtypes = [ctypes.POINTER(ctypes.c_int64),
                                           ctypes.c_size_t]
    lib.axon_start_nrt_profile.restype = ctypes.c_int64
    lib.axon_stop_nrt_profile.argtypes = [ctypes.c_char_p]
    lib.axon_stop_nrt_profile.restype = ctypes.c_int64

    @contextlib.contextmanager
    def _hook(output_dir, device_ids):
        import jax
        jax.devices()
        if device_ids:
            ids = (ctypes.c_int64 * len(device_ids))(*device_ids)
            rc = lib.axon_start_nrt_profile(ids, len(device_ids))
        else:
            rc = lib.axon_start_nrt_profile(None, 0)
        if rc != 0:
            raise RuntimeError(f"axon_start_nrt_profile rc={rc}")
        try:
            yield
        finally:
            n = lib.axon_stop_nrt_profile(str(output_dir).encode())
            print(f"ntff profile: {n} file(s) -> {output_dir}", file=sys.stderr)

    mod = types.ModuleType("antenv.axon_hooks")
    mod.get_axon_ntff_profile_hook = lambda: _hook
    mod.set_axon_ntff_profile_hook = lambda h: None
    import antenv
    antenv.axon_hooks = mod
    sys.modules["antenv.axon_hooks"] = mod


def kernel(**inputs) -> np.ndarray:
    global last_results
    from concourse.bass_utils import run_bass_kernel_spmd

    if "nc" not in _prog_cache:
        _prog_cache["nc"] = _build_program()
    nc = _prog_cache["nc"]

    in_maps = _host_prep(**inputs)
    trace = os.environ.get("DIRATTN_TRACE", "0") == "1"
    if trace:
        _install_ntff_hook()
    res = run_bass_kernel_spmd(nc, in_maps, core_ids=list(range(NCORES)),
                               trace=trace)
    last_results = res
    out = np.empty((B, N, H), np.float32)
    for c in range(NCORES):
        out[:, c * ISH:(c + 1) * ISH, :] = res.results[c]["out"]
    return out



# revision 42
# speedup vs baseline: 1.0040x; 1.0040x over previous
"""DirectionalAttention Trainium2 kernel.

Math (per batch b, query node i, key node j):
    Q/K/V = features @ W{q,k,v} + b{q,k,v}                     [B,N,H]
    dir_enc[b,i,j] = (loc_j - loc_i) @ Wd[0:2] + wind_bi @ Wd[2:4] + bd
    pre[b,i,j,:]  = (Q_bi * K_bj) @ W1a + dir_enc @ W1b + b1   [MLP]
    scores[b,i,j] = relu(pre) @ W2 (+ b2, softmax-invariant -> dropped)
    out[b,i]      = softmax_j(scores) @ V_b

Key restructuring: dir_enc is rank-2 in j (loc has 2 coords), so with
    WdW1   = Wd[0:2] @ W1b   [2,MLP]
    cw1_bi = (-loc_i @ Wd[0:2] + wind_bi @ Wd[2:4] + bd) @ W1b + b1
the whole pre-activation for one i collapses to ONE 66-row matmul:
    pre[m, j] = concat(Q_bi*W1a, WdW1).T @ concat(K_b.T, loc.T)  + cw1_bi[m]
with the per-i bias applied inside the relu (per-partition bias).

Sharding: query axis i split across 8 cores (64 i's per batch per core);
K/V/params replicated; softmax over j stays local. No collectives.
"""

import os
from contextlib import ExitStack

import numpy as np

B, N, H, MLP = 4, 512, 64, 128
NCORES = 8
ISH = N // NCORES            # i's per batch per core = 64
SLOTS = B * ISH              # (b, i) pairs per core = 256
BLK = 128                    # slots per softmax block
NBLK = SLOTS // BLK          # 2
CP = 128                     # padded contraction dim: all matmuls share C=128
DELAY = 8                    # slots the scores matmul trails the main matmul

SD_NAME = os.environ.get("DIRATTN_SD", "bf16")        # hmid/scores-mm dtype
ACTP_N = int(os.environ.get("DIRATTN_ACTP", "5"))    # of 8 relu-pairs on ScalarE
PD_NAME = os.environ.get("DIRATTN_PD", "bf16")       # softmax-P / attn@V dtype
EPI_DELAY = 10               # mains of next block issued before epilogue

_prog_cache = {}


def _build_program():
    import concourse.bass as bass
    import concourse.tile as tile
    from concourse import bacc, mybir
    from concourse.masks import make_identity
    from concourse.tile import add_dep_helper

    f32 = mybir.dt.float32
    SD = mybir.dt.bfloat16 if SD_NAME == "bf16" else f32
    PD = mybir.dt.bfloat16 if PD_NAME == "bf16" else f32
    AF = mybir.ActivationFunctionType
    AL = mybir.AluOpType

    nc = bacc.Bacc("TRN2", target_bir_lowering=False, debug=False,
                   enable_asserts=False)

    d_ft1 = nc.dram_tensor("ft1", (H + 1, B * N), SD, kind="ExternalInput").ap()
    d_fq1 = nc.dram_tensor("fq1", (H + 1, SLOTS), SD, kind="ExternalInput").ap()
    d_locT = nc.dram_tensor("locT", (2, N), f32, kind="ExternalInput").ap()
    d_rhs5 = nc.dram_tensor("rhs5", (5, SLOTS), f32, kind="ExternalInput").ap()
    d_wq1 = nc.dram_tensor("wq1", (H + 1, H), SD, kind="ExternalInput").ap()
    d_wk1 = nc.dram_tensor("wk1", (H + 1, H), SD, kind="ExternalInput").ap()
    d_wv1 = nc.dram_tensor("wv1", (H + 1, H), SD, kind="ExternalInput").ap()
    d_l5 = nc.dram_tensor("lhsT5", (5, MLP), f32, kind="ExternalInput").ap()
    d_w1ax = nc.dram_tensor("w1ax", (CP, MLP), SD, kind="ExternalInput").ap()
    d_w2oh = nc.dram_tensor("w2oh", (MLP, 2 * BLK - 1), SD, kind="ExternalInput").ap()
    d_out = nc.dram_tensor("out", (B, ISH, H), f32, kind="ExternalOutput").ap()

    with tile.TileContext(nc) as tc, ExitStack() as ctx:
        singles = ctx.enter_context(tc.tile_pool(name="singles", bufs=1))
        mm_pool = ctx.enter_context(tc.tile_pool(name="mm", bufs=4, space="PSUM"))
        sc_pool = ctx.enter_context(tc.tile_pool(name="scps", bufs=2, space="PSUM"))
        tr_pool = ctx.enter_context(tc.tile_pool(name="trps", bufs=1, space="PSUM"))
        ov_pool = ctx.enter_context(tc.tile_pool(name="ovps", bufs=1, space="PSUM"))
        lt_pool = ctx.enter_context(tc.tile_pool(name="lt", bufs=7))
        hm_pool = ctx.enter_context(tc.tile_pool(name="hm", bufs=14))
        soft = ctx.enter_context(tc.tile_pool(name="soft", bufs=2))

        def load(name, dram, shape, dt=f32, eng=None):
            t = singles.tile(list(shape), dt, tag=name)
            (eng or nc.sync).dma_start(out=t[:], in_=dram)
            return t

        # split loads across the two hwdge queues; first-needed first
        wk1 = load("wk1", d_wk1, (H + 1, H), SD)
        wq1 = load("wq1", d_wq1, (H + 1, H), SD, eng=nc.scalar)
        ft1 = load("ft1", d_ft1, (H + 1, B * N), SD)
        fq1 = load("fq1", d_fq1, (H + 1, SLOTS), SD, eng=nc.scalar)
        wv1 = load("wv1", d_wv1, (H + 1, H), SD)
        locT = load("locT", d_locT, (2, N), eng=nc.scalar)
        rhs5 = load("rhs5", d_rhs5, (5, SLOTS), eng=nc.scalar)
        l5 = load("l5", d_l5, (5, MLP), eng=nc.scalar)
        w1ax = load("w1ax", d_w1ax, (CP, MLP), SD, eng=nc.scalar)
        w2oh = load("w2oh", d_w2oh, (MLP, 2 * BLK - 1), SD)

        ident = singles.tile([128, 128], PD, tag="ident")
        make_identity(nc, ident[:])

        # --- prep: K^T rows + [1,1,locx,locy] rows per batch -> rhs_b
        # [128, 512].  Row 64 is all-ones and pairs with the per-slot cw1
        # bias row of lt, so the relu needs no bias operand. ---
        rhs_b = []
        for b in range(B):
            rb = singles.tile([CP, N], SD, tag=f"rhsb{b}")
            ps = mm_pool.tile([MLP, N], f32, name="ps", tag="mm")
            nc.tensor.matmul(ps[0:H, :], lhsT=wk1[:],
                             rhs=ft1[:, b * N:(b + 1) * N],
                             start=True, stop=True)
            nc.vector.memset(rb[H:CP, :], 0.0)
            nc.vector.memset(rb[H:H + 1, :], 1.0)
            nc.vector.tensor_copy(rb[0:H, :], ps[0:H, :])
            nc.scalar.copy(rb[96:98, :], locT[:])
            rhs_b.append(rb)

        # --- prep: Q^T for own shard (+ones rows 64-127) -> qt1 [128, 256] ---
        qt1 = singles.tile([CP, SLOTS], f32, tag="qt1")
        nc.vector.memset(qt1[H:CP, :], 1.0)
        psq = mm_pool.tile([MLP, N], f32, name="psq", tag="mm")
        nc.tensor.matmul(psq[0:H, 0:SLOTS], lhsT=wq1[:], rhs=fq1[:],
                         start=True, stop=True)
        nc.vector.tensor_copy(qt1[0:H, :], psq[0:H, 0:SLOTS])

        # --- prep: V_b as 4x [128 j, 64 h] tiles per batch ---
        vsb = {}
        for b in range(B):
            for jc in range(4):
                vt = singles.tile([128, H], PD, tag=f"v{b}_{jc}")
                ps = mm_pool.tile([MLP, N], f32, name="ps", tag="mm")
                nc.tensor.matmul(
                    ps[:, 0:H],
                    lhsT=ft1[:, b * N + jc * 128: b * N + (jc + 1) * 128],
                    rhs=wv1[:], start=True, stop=True)
                nc.vector.tensor_copy(vt[:], ps[:, 0:H])
                vsb[(b, jc)] = vt

        # --- prep: per-(b,i) relu bias cw1 [128 m, 256 slots] ---
        cw1 = singles.tile([MLP, SLOTS], f32, tag="cw1")
        psc = mm_pool.tile([MLP, N], f32, name="psc", tag="mm")
        nc.tensor.matmul(psc[:, 0:SLOTS], lhsT=l5[:], rhs=rhs5[:],
                         start=True, stop=True)
        nc.vector.tensor_copy(cw1[:], psc[:, 0:SLOTS])

        # --- main loop (flat over all 256 slots, software-pipelined) ---
        # The scores matmul for slot p issues after the main matmul for
        # slot p+DELAY, so PE never stalls on the relu.  Each block's
        # softmax+attn@V epilogue is issued EPI_DELAY mains into the next
        # block so the PE keeps streaming mains while DVE does softmax.
        hms = {}
        mains = {}
        scoreses = {}
        scts = {}
        Ss = {}
        lts = {}

        def issue_scores(g):
            blk, p = g // BLK, g % BLK
            si = nc.tensor.matmul(scts[blk][:],
                                  lhsT=w2oh[:, BLK - 1 - p: 2 * BLK - 1 - p],
                                  rhs=hms.pop(g)[:],
                                  start=(p == 0), stop=(p == BLK - 1))
            scoreses[g] = si
            if g + DELAY in mains:
                add_dep_helper(si.ins, mains[g + DELAY].ins, sync=False,
                               reason="pipeline: scores_p after main_p+D")
            if p == BLK - 1:
                nc.vector.tensor_copy(Ss[blk][:], scts[blk][:])

        def build_lt(g):
            lt = lt_pool.tile([CP, MLP], SD, tag="lt")
            nc.vector.tensor_scalar_mul(lt[:], w1ax[:], qt1[:, g:g + 1])
            lts[g] = lt

        def epilogue(blk):
            # softmax over j for 128 (b,i) rows; P left unnormalized and
            # the 1/sum folded into the output copy (per-i scale).
            S = Ss.pop(blk)
            nmx = soft.tile([BLK, 1], f32, tag="nmx")
            nc.vector.tensor_reduce(nmx[:], S[:], axis=mybir.AxisListType.X,
                                    op=AL.max, negate=True)
            P = soft.tile([BLK, N], PD, tag="P")
            sm = soft.tile([BLK, 1], f32, tag="sm")
            nc.scalar.activation(P[:], S[:], AF.Exp, bias=nmx[:, 0:1],
                                 scale=1.0, accum_out=sm[:])
            rs = soft.tile([BLK, 1], f32, tag="rs")
            nc.vector.reciprocal(rs[:], sm[:])

            # attn @ V: transpose P, contract over j, scale by 1/sum
            atT = []
            for jc in range(4):
                tp = tr_pool.tile([128, 128], PD, tag="tr")
                nc.tensor.transpose(tp[:], P[:, jc * 128:(jc + 1) * 128],
                                    ident[:])
                ts = soft.tile([128, 128], PD, tag=f"atT{jc}")
                nc.vector.tensor_copy(ts[:], tp[:])
                atT.append(ts)
            for h2 in range(2):
                b = blk * 2 + h2
                ov = ov_pool.tile([ISH, H], f32, tag="ov")
                for jc in range(4):
                    nc.tensor.matmul(ov[:],
                                     lhsT=atT[jc][:, h2 * ISH:(h2 + 1) * ISH],
                                     rhs=vsb[(b, jc)][:],
                                     start=(jc == 0), stop=(jc == 3))
                ob = soft.tile([ISH, H], f32, tag="ob")
                nc.vector.tensor_scalar_mul(ob[:], ov[:],
                                            rs[h2 * ISH:(h2 + 1) * ISH, 0:1])
                nc.sync.dma_start(out=d_out[b, :, :], in_=ob[:])

        LOOKA = 4
        for g in range(LOOKA):
            build_lt(g)
        for g in range(SLOTS):
            blk, p = g // BLK, g % BLK
            b = g // ISH
            if p == 0:
                Ss[blk] = soft.tile([BLK, N], f32, name="S", tag="S")
                scts[blk] = sc_pool.tile([BLK, N], f32, name="sc", tag="sc")
            if g + LOOKA < SLOTS:
                build_lt(g + LOOKA)
            lt = lts.pop(g)

            ps = mm_pool.tile([MLP, N], f32, name="ps", tag="mm")
            mi = nc.tensor.matmul(ps[:], lhsT=lt[:], rhs=rhs_b[b][:],
                                  start=True, stop=True)
            mains[g] = mi
            if g - DELAY - 1 in scoreses:
                add_dep_helper(mi.ins, scoreses[g - DELAY - 1].ins,
                               sync=False,
                               reason="pipeline: main_p after scores")

            hm = hm_pool.tile([MLP, N], SD, name="hm", tag="hm")
            if p % 8 < ACTP_N:
                nc.scalar.activation(hm[:], ps[:], AF.Relu,
                                     bias=cw1[:, g:g + 1], scale=1.0)
            else:
                nc.vector.tensor_scalar(hm[:], ps[:], cw1[:, g:g + 1], 0.0,
                                        AL.add, AL.max)
            hms[g] = hm
            if g >= DELAY:
                issue_scores(g - DELAY)
            if p == EPI_DELAY and blk > 0:
                epilogue(blk - 1)
        for q in range(SLOTS - DELAY, SLOTS):
            issue_scores(q)
        epilogue(NBLK - 1)

    nc.compile()
    return nc


def _host_prep(features, wind_data, loc_feature, Wq, bq, Wk, bk, Wv, bv,
               Wd, bd, W1, b1, W2, b2):
    f32 = np.float32
    f = np.ascontiguousarray(features, dtype=f32)
    W1a, W1b = W1[:H].astype(f32), W1[H:].astype(f32)
    WdW1 = (Wd[0:2].astype(f32) @ W1b)
    Wd24W1 = (Wd[2:4].astype(f32) @ W1b)
    g = bd.astype(f32) @ W1b + b1.astype(f32)

    fT = f.reshape(B * N, H).T
    ones = np.ones((1, B * N), f32)
    ft1 = np.ascontiguousarray(np.vstack([fT, ones]))
    locT = np.ascontiguousarray(loc_feature.T.astype(f32))
    wind = np.asarray(wind_data, dtype=f32)

    if SD_NAME == "bf16":
        import ml_dtypes
        sd = ml_dtypes.bfloat16
    else:
        sd = f32
    ft1_sd = ft1.astype(sd)
    w2oh = np.zeros((MLP, 2 * BLK - 1), f32)
    w2oh[:, BLK - 1] = W2[:, 0]
    w2oh = w2oh.astype(sd)

    shared = {
        "ft1": ft1_sd,
        "locT": locT,
        "wq1": np.ascontiguousarray(np.vstack([Wq, bq[None]]).astype(sd)),
        "wk1": np.ascontiguousarray(np.vstack([Wk, bk[None]]).astype(sd)),
        "wv1": np.ascontiguousarray(np.vstack([Wv, bv[None]]).astype(sd)),
        "lhsT5": np.ascontiguousarray(
            np.vstack([-WdW1, Wd24W1, g[None]]).astype(f32)),
        "w1ax": np.ascontiguousarray(np.vstack(
            [W1a, np.zeros((32, MLP), f32), WdW1,
             np.zeros((CP - 98, MLP), f32)]).astype(sd)),
        "w2oh": w2oh,
    }
    in_maps = []
    for c in range(NCORES):
        i0, i1 = c * ISH, (c + 1) * ISH
        fq1 = np.concatenate(
            [ft1[:, b * N + i0: b * N + i1] for b in range(B)], axis=1)
        rhs5 = np.vstack([
            np.concatenate([locT[:, i0:i1]] * B, axis=1),
            np.concatenate([wind[b, i0:i1, :].T for b in range(B)], axis=1),
            np.ones((1, SLOTS), f32),
        ])
        in_maps.append({**shared,
                        "fq1": np.ascontiguousarray(fq1.astype(sd)),
                        "rhs5": np.ascontiguousarray(rhs5.astype(f32))})
    return in_maps


last_results = None


def _install_ntff_hook():
    """Provide antenv.axon_hooks (absent in this image) so that
    run_bass_kernel_spmd(trace=True) can capture NTFF profiles via the
    injected libaxon_pjrt.so C ABI."""
    import sys
    if "antenv.axon_hooks" in sys.modules:
        return
    import contextlib
    import ctypes
    import types

    so_path = "/opt/axon/libaxon_pjrt.so"
    try:
        lib = ctypes.CDLL(so_path)
        lib.axon_start_nrt_profile
    except (OSError, AttributeError):
        return
    lib.axon_start_nrt_profile.argtypes = [ctypes.POINTER(ctypes.c_int64),
                                           ctypes.c_size_t]
    lib.axon_start_nrt_profile.restype = ctypes.c_int64
    lib.axon_stop_nrt_profile.argtypes = [ctypes.c_char_p]
    lib.axon_stop_nrt_profile.restype = ctypes.c_int64

    @contextlib.contextmanager
    def _hook(output_dir, device_ids):
        import jax
        jax.devices()
        if device_ids:
            ids = (ctypes.c_int64 * len(device_ids))(*device_ids)
            rc = lib.axon_start_nrt_profile(ids, len(device_ids))
        else:
            rc = lib.axon_start_nrt_profile(None, 0)
        if rc != 0:
            raise RuntimeError(f"axon_start_nrt_profile rc={rc}")
        try:
            yield
        finally:
            n = lib.axon_stop_nrt_profile(str(output_dir).encode())
            print(f"ntff profile: {n} file(s) -> {output_dir}", file=sys.stderr)

    mod = types.ModuleType("antenv.axon_hooks")
    mod.get_axon_ntff_profile_hook = lambda: _hook
    mod.set_axon_ntff_profile_hook = lambda h: None
    import antenv
    antenv.axon_hooks = mod
    sys.modules["antenv.axon_hooks"] = mod


def kernel(**inputs) -> np.ndarray:
    global last_results
    from concourse.bass_utils import run_bass_kernel_spmd

    if "nc" not in _prog_cache:
        _prog_cache["nc"] = _build_program()
    nc = _prog_cache["nc"]

    in_maps = _host_prep(**inputs)
    trace = os.environ.get("DIRATTN_TRACE", "0") == "1"
    if trace:
        _install_ntff_hook()
    res = run_bass_kernel_spmd(nc, in_maps, core_ids=list(range(NCORES)),
                               trace=trace)
    last_results = res
    out = np.empty((B, N, H), np.float32)
    for c in range(NCORES):
        out[:, c * ISH:(c + 1) * ISH, :] = res.results[c]["out"]
    return out

